# revision 29
# baseline (speedup 1.0000x reference)
"""Category-specific MLP (MoE-style routing) on 8 Trainium2 NeuronCores.

Strategy (host-routed expert/data parallel):
  - Host groups the 64 samples by cat_id into per-core work so every core
    gets exactly 8 samples (2048 tokens): token-balanced.
  - Same-cat samples are paired into 512-token "runs" where possible so a
    run does larger matmuls and loads its weight bank once. The per-core
    run profile (npair pair-runs + singles) is chosen from cat_ids and is
    identical on all cores, so one SPMD program serves all 8 cores; the
    program is (re)built per profile and cached.
  - Host gathers each run's weight bank W_l[cat] into a per-core DRAM
    input. Weights/activations are bf16 on chip (fp32 PSUM accumulate),
    final output fp32.
  - Activations live in transposed layout [D, tok]; each layer computes
    out_T = W_l.T @ h_T via matmul(lhsT=W tile, rhs=h_T tile), so layers
    chain on the tensor engine with no transposes. Host transposes x on
    the way in and the output on the way out.
"""

import numpy as np
from contextlib import ExitStack

import ml_dtypes

import concourse.bass as bass
import concourse.mybir as mybir
import concourse.tile as tile
from concourse import bacc
from concourse.bass_utils import run_bass_kernel_spmd

P = 128          # SBUF partitions
D = 1024         # model dim (in = hidden = out)
KT = D // P      # 8 k-tiles per dim
TOK = 256        # tokens per sample
S = 8            # samples per core
L = 4            # layers
NCORES = 8

ACT_DT = mybir.dt.bfloat16  # on-chip activation dtype
W_DT = mybir.dt.bfloat16    # on-chip weight dtype
ACT_NP = ml_dtypes.bfloat16
W_NP = ml_dtypes.bfloat16

# Filled by kernel() with the BassKernelResults of the last run (for tests).
LAST_RESULT = None
_PROGRAM_CACHE = {}


def plan(cat_ids):
    """Pick per-core sample order and the uniform run profile.

    Returns (order, npair): order is a [64] array of sample indices; core c
    owns order[8c:8c+8]. The first 2*npair samples of each core form npair
    same-cat pairs (512-token runs); the rest are single-sample runs.
    """
    cat_ids = np.asarray(cat_ids).astype(np.int64)
    by_cat = {}
    for i, c in enumerate(cat_ids.tolist()):
        by_cat.setdefault(c, []).append(i)
    pairs, singles = [], []
    for c in sorted(by_cat):
        lst = by_cat[c]
        for i in range(len(lst) // 2):
            pairs.append((lst[2 * i], lst[2 * i + 1]))
        if len(lst) % 2:
            singles.append(lst[-1])
    npair = min(S // 2, len(pairs) // NCORES)
    for a, b in pairs[npair * NCORES:]:
        singles.extend([a, b])
    pairs = pairs[:npair * NCORES]
    nsing = S - 2 * npair
    order = []
    for c in range(NCORES):
        csing = singles[c * nsing:(c + 1) * nsing]
        # one single first (smaller startup x-DMA), pairs, singles, so the
        # first and last runs are the short ones
        if csing:
            order.append(csing[0])
        for a, b in pairs[c * npair:(c + 1) * npair]:
            order.extend([a, b])
        order.extend(csing[1:])
    return np.asarray(order), npair


def _run_toks(npair):
    nsing = S - 2 * npair
    if nsing == 0:
        return [2 * TOK] * npair
    return [TOK] + [2 * TOK] * npair + [TOK] * (nsing - 1)


def build_program(reps=1, npair=0, mode="full", dual_dma=True, h_gens=3,
                  w_banks=4, ps_bufs=8):
    """One SPMD program for all 8 cores: R runs x 4 layers.

    reps>1 wraps the computation in a hardware loop (only used for
    wall-clock slope timing in the test harness; grading uses reps=1).
    mode: "full" (graded), "dma_only" / "compute_only" for bottleneck
    attribution in the test harness.
    """
    toks = _run_toks(npair)
    R = len(toks)
    offs = np.concatenate([[0], np.cumsum(toks)])

    nc = bacc.Bacc("TRN2", target_bir_lowering=False, debug=False,
                   num_devices=NCORES)
    xT = nc.dram_tensor("xT", [D, S * TOK], ACT_DT, kind="ExternalInput")
    wg = nc.dram_tensor("wg", [R, L, D, D], W_DT, kind="ExternalInput")
    bg = nc.dram_tensor("bg", [L, R, D], mybir.dt.float32, kind="ExternalInput")
    outT = nc.dram_tensor("outT", [D, S * TOK], mybir.dt.float32,
                          kind="ExternalOutput")

    xv = xT.ap().rearrange("(k p) n -> p k n", p=P)
    ov = outT.ap().rearrange("(k p) n -> p k n", p=P)
    bv = bg.ap().rearrange("l r (t p) -> p (l r t)", p=P)

    silu = mybir.ActivationFunctionType.Silu

    with tile.TileContext(nc) as tc, ExitStack() as ctx:
        wpool = ctx.enter_context(
            tc.tile_pool(name="w",
                         bufs=KT * (5 if mode == "compute_only" else w_banks)))
        # 3 live bf16 generations; deeper buffering measured slower (DMA
        # prefetch contends with PE SBUF reads)
        hpool = ctx.enter_context(tc.tile_pool(name="h", bufs=h_gens * KT))
        opool = ctx.enter_context(
            tc.tile_pool(name="o", bufs=(2 if h_gens <= 3 else 3) * KT))
        ppool = ctx.enter_context(
            tc.tile_pool(name="ps", bufs=ps_bufs, space="PSUM"))
        cpool = ctx.enter_context(tc.tile_pool(name="c", bufs=1))

        btile = cpool.tile([P, L * R * KT], mybir.dt.float32)
        nc.scalar.dma_start(btile[:], bv[:, :])

        # m-groups run descending, so within a layer the last tile produced
        # is m=0; the next layer then consumes k=0 last (k-order rotation),
        # hiding the previous layer's final activation latency.
        ms = list(reversed(range(KT)))
        ks = list(range(1, KT)) + [0]
        dq = [nc.sync, nc.scalar]

        def body(_iv=None):
            once_w = {}
            qi = 0
            for r in range(R):
                tok, off = toks[r], int(offs[r])
                hin = [hpool.tile([P, 2 * TOK], ACT_DT, tag="acts",
                                  name=f"hin{k}") for k in range(KT)]
                for l in range(L):
                    if mode == "compute_only" and l in once_w:
                        w = once_w[l]
                    else:
                        # one SBUF tile per k-slice so the first matmul only
                        # waits on a 256KB DMA, not the whole 2MB bank
                        w = [wpool.tile([P, D], W_DT, tag="w", name=f"wk{k}")
                             for k in range(KT)]
                        wsrc = wg.ap()[r, l].rearrange("(k p) m -> p k m", p=P)
                        korder = range(KT) if (r == 0 and l == 0) else ks
                        for k in korder:
                            if dual_dma:
                                eng, qi = dq[qi], 1 - qi
                            else:
                                eng = nc.sync
                            eng.dma_start(w[k][:], wsrc[:, k, :])
                            if l == 0:
                                if dual_dma:
                                    eng, qi = dq[qi], 1 - qi
                                eng.dma_start(hin[k][:, :tok],
                                              xv[:, k, off:off + tok])
                        if mode == "compute_only":
                            once_w[l] = w
                    last = l == L - 1
                    if last:
                        hout = [opool.tile([P, 2 * TOK], mybir.dt.float32,
                                           tag="outs", name=f"ho{k}")
                                for k in range(KT)]
                    else:
                        hout = [hpool.tile([P, 2 * TOK], ACT_DT, tag="acts",
                                           name=f"hu{k}") for k in range(KT)]
                    if mode == "dma_only":
                        hin = hout if last else hin
                        continue

                    def epilogue(m, ps):
                        col = (l * R + r) * KT + m
                        if last:
                            # bias-add on DVE: keeps ACT running only Silu
                            # (no activation-table switches), f32 output.
                            nc.vector.tensor_scalar_add(
                                hout[m][:, :tok], ps[:, :tok],
                                btile[:, col:col + 1])
                        else:
                            nc.scalar.activation(hout[m][:, :tok], ps[:, :tok],
                                                 silu,
                                                 bias=btile[:, col:col + 1])

                    if r == 0 and l == 0:
                        # k-outer: stream behind the very first weight DMAs
                        # instead of idling until the whole bank lands.
                        pss = [ppool.tile([P, 2 * TOK], mybir.dt.float32,
                                          tag="ps", name=f"ps{m}")
                               for m in range(KT)]
                        for j in range(KT):
                            for m in ms:
                                nc.tensor.matmul(pss[m][:, :tok],
                                                 w[j][:, m * P:(m + 1) * P],
                                                 hin[j][:, :tok],
                                                 start=(j == 0),
                                                 stop=(j == KT - 1))
                        for m in ms:
                            epilogue(m, pss[m])
                    else:
                        for m in ms:
                            ps = ppool.tile([P, 2 * TOK], mybir.dt.float32)
                            for j, k in enumerate(ks):
                                nc.tensor.matmul(ps[:, :tok],
                                                 w[k][:, m * P:(m + 1) * P],
                                                 hin[k][:, :tok],
                                                 start=(j == 0),
                                                 stop=(j == KT - 1))
                            epilogue(m, ps)
                    hin = hout
                # in dma_only mode hin is the (unwritten) f32 opool list here;
                # DMA timing is what matters there, not values.
                for k in range(KT):
                    if dual_dma:
                        eng, qi = dq[qi], 1 - qi
                    else:
                        eng = nc.sync
                    eng.dma_start(ov[:, k, off:off + tok], hin[k][:, :tok])

        if reps == 1:
            body()
        else:
            with tc.For_i(0, reps, 1) as iv:
                body(iv)
    nc.compile()
    return nc


def prepare_in_maps(x, cat_ids, Ws, bs, order, npair):
    x = np.asarray(x)
    cat_ids = np.asarray(cat_ids).astype(np.int64)
    toks = _run_toks(npair)
    in_maps = []
    for c in range(NCORES):
        samp = order[c * S:(c + 1) * S]
        xs = np.asarray(x[samp], dtype=np.float32)            # [S, TOK, D]
        xTc = np.ascontiguousarray(xs.reshape(S * TOK, D).T)  # [D, S*TOK]
        # one weight bank per run; run r starts at sample index sum(prev)/TOK
        run_first = np.concatenate([[0], np.cumsum(toks)])[:-1] // TOK
        cats = [int(cat_ids[samp[i]]) for i in run_first]
        wgc = np.stack([np.stack([Ws[l][cat] for l in range(L)])
                        for cat in cats])                     # [R, L, D, D]
        bgc = np.stack([np.stack([bs[l][cat] for cat in cats])
                        for l in range(L)])                   # [L, R, D]
        in_maps.append({
            "xT": xTc.astype(ACT_NP),
            "wg": np.ascontiguousarray(wgc).astype(W_NP),
            "bg": np.ascontiguousarray(bgc).astype(np.float32),
        })
    return in_maps


def finish_output(results, order, B):
    out = np.empty((B, TOK, D), np.float32)
    for c in range(NCORES):
        outTc = results[c]["outT"]                  # [D, S*TOK] f32
        out[order[c * S:(c + 1) * S]] = outTc.T.reshape(S, TOK, D)
    return out


def kernel(x, cat_ids, W1, b1, W2, b2, W3, b3, W4, b4):
    global LAST_RESULT
    cat_ids = np.asarray(cat_ids).astype(np.int64)
    Ws = [np.asarray(w, dtype=np.float32) for w in (W1, W2, W3, W4)]
    bs = [np.asarray(b, dtype=np.float32) for b in (b1, b2, b3, b4)]
    x = np.asarray(x, dtype=np.float32)
    B = x.shape[0]

    order, npair = plan(cat_ids)
    in_maps = prepare_in_maps(x, cat_ids, Ws, bs, order, npair)

    if npair not in _PROGRAM_CACHE:
        _PROGRAM_CACHE[npair] = build_program(npair=npair)
    nc = _PROGRAM_CACHE[npair]

    res = run_bass_kernel_spmd(nc, in_maps, list(range(NCORES)))
    LAST_RESULT = res
    return finish_output(res.results, order, B)
